# revision 15
# baseline (speedup 1.0000x reference)
"""vq_codebook kernel for trn2: cosine-sim argmax over K=65536 codes + codebook gather.

Strategy: shard K across 8 cores. Per core (slab Kc=8192):
  - fp16 matmul screen: sims = targ @ (W * diag(1/colnorm))  (row norms don't
    affect the argmax over k; eps is absorbed by the host-side margin check)
  - PE -> PSUM fp32; ACT copies PSUM -> SBUF fp16; DVE computes, per 128-row
    block, an elementwise max over the 8 interleaved planes sims[:, j*8+c]
    (c = k mod 8) in 3 tensor_max ops, then max8/max_index over the 1024-wide
    root -> top position j* and top-2 root values.
  - candidates k in [8*j*, 8*j*+8) are contiguous: one indirect DMA per block
    gathers the 8 candidate codebook rows (fp32, exact) from the W^T slab.
Host: exactly rescores the 8 candidates per core (the gathered rows ARE the
codebook vectors) in float64, picks the global winner among 64 candidates,
and fully recomputes any row where a screened-out code could beat the best
candidate (second root value + error band >= best candidate sim).
"""

import os
import sys

import numpy as np

for _p in ("/opt/trn_rl_repo", "/root/.axon_site/_ro/trn_rl_repo"):
    if os.path.isdir(_p) and _p not in sys.path:
        sys.path.append(_p)

import concourse.bass as bass
import concourse.bass_isa as bass_isa
import concourse.tile as tile
from concourse import bacc, mybir
from concourse.bass import IndirectOffsetOnAxis
from concourse.bass_utils import run_bass_kernel_spmd

P = 128
B, D, K, NCORES = 8192, 256, 65536, 8
KC = K // NCORES  # 8192 per-core codebook slab
NCH = 8           # interleave factor: candidate group = k mod NCH
EPS = 1e-7

# cosine-unit bound on |fp16 screen - exact| incl. fp16 sims quantization
# (measured 2.6e-4 worst-case on seed-0; 3x safety)
BAND = 8.0e-4

F32 = mybir.dt.float32
F16 = mybir.dt.float16
U32 = mybir.dt.uint32
AF = mybir.ActivationFunctionType
ALU = mybir.AluOpType


def build_core_kernel(nc, b=B, d=D, kc=KC, qw=2048, pck=512):
    """Emit the per-core kernel. b: batch rows, d: feature dim (must be 256),
    kc: per-core codebook columns, qw: PSUM quarter width, pck: prologue
    chunk width."""
    assert d == 2 * P
    mb = b // P           # number of 128-row blocks
    nq = kc // qw         # PSUM quarters per block
    nj = kc // NCH        # root width (candidate-group count)

    tT = nc.dram_tensor("tT", [d, b], F32, kind="ExternalInput")
    w = nc.dram_tensor("w", [d, kc], F32, kind="ExternalInput")
    wT = nc.dram_tensor("wT", [kc, d], F32, kind="ExternalInput")
    g1_d = nc.dram_tensor("g1", [P, mb], F32, kind="ExternalOutput")
    g2_d = nc.dram_tensor("g2", [P, mb], F32, kind="ExternalOutput")
    jpos_d = nc.dram_tensor("jpos", [P, mb], U32, kind="ExternalOutput")
    rows_d = nc.dram_tensor("rows8", [b, NCH * d], F32, kind="ExternalOutput")
    invb = nc.dram_tensor("invb", [1, kc], F32)  # internal bounce for 1/colnorm

    with tile.TileContext(nc) as tc:
        with (
            tc.tile_pool(name="persist", bufs=1) as persist,
            tc.tile_pool(name="stage", bufs=max(2, 2048 // pck)) as stage,
            tc.tile_pool(name="sq", bufs=2 if pck <= 512 else 1) as sqp,
            tc.tile_pool(name="cn", bufs=1) as cnp,
            tc.tile_pool(name="sims", bufs=2) as simsp,
            tc.tile_pool(name="tree", bufs=1) as treep,
            tc.tile_pool(name="small", bufs=4) as smallp,
            tc.tile_pool(name="rowout", bufs=2) as rowp,
            tc.tile_pool(name="psum", bufs=2, space="PSUM") as psump,
        ):
            # ---- persistent tiles ----
            Tn = persist.tile([P, 2 * b], F16)    # targ^T, fp16
            Wn = persist.tile([P, 2 * kc], F16)   # col-normalized W, fp16
            g1w = persist.tile([P, mb], F32)
            g2w = persist.tile([P, mb], F32)
            jw = persist.tile([P, mb], U32)

            # ---- prologue A: column norms of W (chunked) ----
            for c in range(kc // pck):
                sl = slice(c * pck, (c + 1) * pck)
                wlo = stage.tile([P, pck], F32, tag="wst")
                whi = stage.tile([P, pck], F32, tag="wst")
                nc.sync.dma_start(out=wlo[:], in_=w[0:P, sl])
                nc.sync.dma_start(out=whi[:], in_=w[P : 2 * P, sl])
                sqa = sqp.tile([P, pck], F32, tag="sqa")
                sqb = sqp.tile([P, pck], F32, tag="sqb")
                nc.scalar.activation(sqa[:], wlo[:], AF.Square)
                nc.scalar.activation(sqb[:], whi[:], AF.Square)
                wss = sqp.tile([P, pck], F32, tag="wss")
                nc.vector.tensor_add(wss[:], sqa[:], sqb[:])
                pr = sqp.tile([P, pck], F32, tag="pr")
                nc.gpsimd.partition_all_reduce(
                    pr[:], wss[:], channels=P, reduce_op=bass_isa.ReduceOp.add
                )
                # row 0 holds colnorm^2 for this chunk; park it in DRAM
                nc.sync.dma_start(out=invb[0:1, sl], in_=pr[0:1, :])

            # reshape [1, kc] -> [128, kc/128] (k = p*(kc/128) + j); rsqrt+newton
            jwid = kc // P
            cn2 = cnp.tile([P, jwid], F32)
            nc.sync.dma_start(
                out=cn2[:], in_=invb[:].rearrange("o (p j) -> (o p) j", p=P)
            )
            srt = cnp.tile([P, jwid], F32)
            nc.scalar.activation(srt[:], cn2[:], AF.Sqrt)
            u0 = cnp.tile([P, jwid], F32)
            nc.vector.reciprocal(u0[:], srt[:])
            uu = cnp.tile([P, jwid], F32)
            nc.vector.tensor_mul(uu[:], u0[:], u0[:])
            nc.vector.tensor_mul(uu[:], uu[:], cn2[:])
            nc.vector.tensor_scalar(uu[:], uu[:], -0.5, 1.5, op0=ALU.mult, op1=ALU.add)
            u1 = cnp.tile([P, jwid], F32)
            nc.vector.tensor_mul(u1[:], u0[:], uu[:])
            nc.sync.dma_start(
                out=invb[:].rearrange("o (p j) -> (o p) j", p=P), in_=u1[:]
            )

            # ---- prologue B: cast targ^T -> fp16 (chunked) ----
            tck = min(pck, b)
            for c in range(b // tck):
                sl = slice(c * tck, (c + 1) * tck)
                tlo = stage.tile([P, tck], F32, tag="tst")
                thi = stage.tile([P, tck], F32, tag="tst")
                nc.sync.dma_start(out=tlo[:], in_=tT[0:P, sl])
                nc.sync.dma_start(out=thi[:], in_=tT[P : 2 * P, sl])
                nc.vector.tensor_copy(Tn[:, c * tck : (c + 1) * tck], tlo[:])
                nc.vector.tensor_copy(Tn[:, b + c * tck : b + (c + 1) * tck], thi[:])

            # ---- prologue C: Wn = W * diag(1/colnorm), fp16 (chunked) ----
            for c in range(kc // pck):
                sl = slice(c * pck, (c + 1) * pck)
                wlo = stage.tile([P, pck], F32, tag="wst2")
                whi = stage.tile([P, pck], F32, tag="wst2")
                nc.sync.dma_start(out=wlo[:], in_=w[0:P, sl])
                nc.sync.dma_start(out=whi[:], in_=w[P : 2 * P, sl])
                icb = stage.tile([P, pck], F32, tag="icb")
                nc.sync.dma_start(out=icb[:], in_=invb[0:1, sl].to_broadcast([P, pck]))
                nc.vector.tensor_mul(Wn[:, c * pck : (c + 1) * pck], wlo[:], icb[:])
                nc.vector.tensor_mul(
                    Wn[:, kc + c * pck : kc + (c + 1) * pck], whi[:], icb[:]
                )

            # view of the W^T slab as candidate groups of NCH consecutive rows
            wT_g = wT[:].rearrange("(a e) d -> a (e d)", e=NCH)

            # ---- main loop over 128-row blocks ----
            for m in range(mb):
                S = simsp.tile([P, kc], F16)
                for q in range(nq):
                    pq = psump.tile([P, qw], F32, space="PSUM")
                    for i in range(2):
                        lhsT = Tn[:, i * b + m * P : i * b + (m + 1) * P]
                        for cc in range(qw // 512):
                            k0 = q * qw + cc * 512
                            nc.tensor.matmul(
                                out=pq[:, cc * 512 : (cc + 1) * 512],
                                lhsT=lhsT,
                                rhs=Wn[:, i * kc + k0 : i * kc + k0 + 512],
                                start=(i == 0),
                                stop=(i == 1),
                            )
                    nc.scalar.activation(
                        S[:, q * qw : (q + 1) * qw], pq[:], AF.Copy, bias=0.0
                    )

                # elementwise max over the NCH=8 interleaved planes (c = k%8)
                S3 = S[:].rearrange("p (j c) -> p j c", c=NCH)
                t1 = treep.tile([P, nj * 4], F16, tag="t1")
                t1v = t1[:].rearrange("p (j c) -> p j c", c=4)
                nc.vector.tensor_max(t1v[:, :, :], S3[:, :, 0:4], S3[:, :, 4:8])
                t2 = treep.tile([P, nj * 2], F16, tag="t2")
                t2v = t2[:].rearrange("p (j c) -> p j c", c=2)
                nc.vector.tensor_max(t2v[:, :, :], t1v[:, :, 0:2], t1v[:, :, 2:4])
                root = treep.tile([P, nj], F16, tag="root")
                nc.vector.tensor_max(root[:], t2v[:, :, 0], t2v[:, :, 1])

                r8 = smallp.tile([P, 8], F16, tag="r8")
                nc.vector.max(out=r8[:], in_=root[:])
                j8 = smallp.tile([P, 8], U32, tag="j8")
                nc.vector.max_index(out=j8[:], in_max=r8[:], in_values=root[:])
                nc.vector.tensor_copy(jw[:, m : m + 1], j8[:, 0:1])
                nc.vector.tensor_copy(g1w[:, m : m + 1], r8[:, 0:1])
                nc.vector.tensor_copy(g2w[:, m : m + 1], r8[:, 1:2])

                rowt = rowp.tile([P, NCH * d], F32)
                nc.gpsimd.indirect_dma_start(
                    out=rowt[:],
                    out_offset=None,
                    in_=wT_g,
                    in_offset=IndirectOffsetOnAxis(ap=jw[:, m : m + 1], axis=0),
                )
                nc.sync.dma_start(out=rows_d[m * P : (m + 1) * P, :], in_=rowt[:])

            nc.sync.dma_start(out=g1_d[:], in_=g1w[:])
            nc.sync.dma_start(out=g2_d[:], in_=g2w[:])
            nc.sync.dma_start(out=jpos_d[:], in_=jw[:])

    nc.compile()
    return nc


_CACHE = {}
LAST_RESULT = None
LAST_AMB = -1


def _get_nc():
    if "nc" not in _CACHE:
        nc = bacc.Bacc(
            "TRN2", target_bir_lowering=False, debug=False, enable_asserts=False
        )
        build_core_kernel(nc)
        _CACHE["nc"] = nc
    return _CACHE["nc"]


def _unpack_vec(arr):
    # [128, mb] with b = m*128 + p  ->  [b]
    return np.ascontiguousarray(arr.T).ravel()


def kernel(targ: np.ndarray, W: np.ndarray) -> np.ndarray:
    assert targ.shape == (B, D) and W.shape == (D, K)
    targ = np.ascontiguousarray(targ, dtype=np.float32)
    W = np.ascontiguousarray(W, dtype=np.float32)
    nc = _get_nc()

    tT = np.ascontiguousarray(targ.T)
    in_maps = []
    for c in range(NCORES):
        wslab = np.ascontiguousarray(W[:, c * KC : (c + 1) * KC])
        in_maps.append({"tT": tT, "w": wslab, "wT": np.ascontiguousarray(wslab.T)})

    global LAST_RESULT
    LAST_RESULT = run_bass_kernel_spmd(nc, in_maps, list(range(NCORES)))
    res = LAST_RESULT.results

    g2 = np.stack([_unpack_vec(r["g2"]) for r in res])            # [NC, B]
    jpos = np.stack([_unpack_vec(r["jpos"]) for r in res])        # [NC, B]
    rows8 = np.stack([r["rows8"].reshape(B, NCH, D) for r in res])  # [NC,B,8,D]

    # exact rescore of the NCORES*NCH candidates per row (float64)
    t64 = targ.astype(np.float64)
    rown = np.linalg.norm(t64, axis=1)
    cand = rows8.transpose(1, 0, 2, 3).reshape(B, NCORES * NCH, D)  # k-ordered
    c64 = cand.astype(np.float64)
    dots = np.einsum("bkd,bd->bk", c64, t64)
    cnorm = np.linalg.norm(c64, axis=2)
    sims = dots / (rown[:, None] * cnorm + EPS)
    best_c = np.argmax(sims, axis=1)                 # first max = smallest k
    best_cos = sims[np.arange(B), best_c]
    out = cand[np.arange(B), best_c, :].astype(np.float32)

    # any non-candidate code k on core c has screen value <= g2[c,b], hence
    # exact cosine <= g2[c,b]/||t_b|| + BAND.  Accept iff best candidate beats
    # that bound.
    bound = g2.max(axis=0) / rown + BAND
    # also guard candidate-vs-candidate near-ties (fp32 reference could order
    # them differently than our f64 rescore)
    s_sorted = np.sort(sims, axis=1)
    cand_tie = (s_sorted[:, -1] - s_sorted[:, -2]) < 1e-6
    amb = np.where((best_cos < bound) | cand_tie)[0]
    global LAST_AMB
    LAST_AMB = len(amb)
    if len(amb):
        col_nm = np.linalg.norm(W, axis=0)
        t_amb = targ[amb]
        s = (t_amb @ W) / (
            np.linalg.norm(targ[amb], axis=1)[:, None] * col_nm[None, :] + EPS
        )
        k_star = np.argmax(s, axis=1)
        out[amb] = W[:, k_star].T
    return out
